# revision 9
# baseline (speedup 1.0000x reference)
"""Trainium2 Bass kernel for 3x3 VALID conv (NCHW, stride 1) via block-Toeplitz GEMM.

Full input (64, 8, 256, 256) f32 + filter (8, 8, 3, 3) -> output (64, 8, 254, 254).
Data-parallel over batch: 8 images per NeuronCore, 8 cores.

Per-core formulation: for a block of G=14 output rows, load input rows
i0..i0+15 of all 8 channels into SBUF partitions (c*16+h) -> K=128.
For each kernel column offset s in {0,1,2} do one accumulating matmul with a
host-precomputed block-Toeplitz weight lhsT[c*16+h, m*14+i] = filter[m, c,
h-i, s], producing PSUM [112=(m,i), 2 images * 254 pixels = 508].  N=508 >=
256 so float32r (full fp32 precision) streams at 1 cycle/row on the PE.
"""

import numpy as np

import concourse.bacc as bacc
import concourse.bass as bass
import concourse.mybir as mybir
import concourse.tile as tile
from concourse import bass_utils

F32 = mybir.dt.float32
F32R = mybir.dt.float32r

N_CORES = 8
N_LOC = 8  # images per core
C, H, W = 8, 256, 256
M, R, S = 8, 3, 3
HO, WO = H - R + 1, W - S + 1  # 254, 254
G = 14  # output rows per full block
NBLK = 18  # full blocks (rows 0..251)
GT = 2  # tail block output rows (252, 253)

_CACHE = {}


def _round_fp32r(a):
    """Round fp32 array to float32r (12 explicit mantissa bits, RNE)."""
    a = np.ascontiguousarray(a, dtype=np.float32)
    try:
        from neuron_dtypes._impl import fp32r as _fp32r

        u = a.reshape(-1).view(np.uint32)
        r = np.asarray(_fp32r.cast_fp32_to_fp32r(len(u), u), np.uint32)
        return r.view(np.float32).reshape(a.shape)
    except Exception:
        u = a.reshape(-1).view(np.uint32).astype(np.uint64)
        lsb = (u >> 12) & 1
        r = ((u + 0x7FF + lsb) & 0xFFFFF000).astype(np.uint32)
        return r.view(np.float32).reshape(a.shape)


def _toeplitz_weights(f, g):
    """lhsT[c*(g+2)+h, s, m*g+i] = f[m, c, h-i, s] for 0 <= h-i < 3 else 0."""
    hh = g + 2
    out = np.zeros((C * hh, S, M * g), np.float32)
    for c in range(C):
        for h in range(hh):
            for m in range(M):
                for i in range(g):
                    r = h - i
                    if 0 <= r < R:
                        out[c * hh + h, :, m * g + i] = f[m, c, r, :]
    return out


def _build_program():
    nc = bacc.Bacc("TRN2", target_bir_lowering=False, debug=False)
    x = nc.dram_tensor("x", [N_LOC, C, H, W], F32R, kind="ExternalInput").ap()
    w = nc.dram_tensor("w", [C * (G + 2), S, M * G], F32R, kind="ExternalInput").ap()
    wt = nc.dram_tensor("wt", [C * (GT + 2), S, M * GT], F32R, kind="ExternalInput").ap()
    y = nc.dram_tensor("y", [N_LOC, M, HO, WO], F32, kind="ExternalOutput").ap()

    x_v = x.rearrange("n c h w -> c h n w")
    y_v = y.rearrange("n m i j -> m i n j")

    with tile.TileContext(nc) as tc:
        with (
            tc.tile_pool(name="wpool", bufs=1) as wpool,
            tc.tile_pool(name="xpool", bufs=8) as xpool,
            tc.tile_pool(name="opool", bufs=8) as opool,
            tc.tile_pool(name="psum", bufs=6, space=bass.MemorySpace.PSUM) as pspool,
        ):
            wtile = wpool.tile([C * (G + 2), S, M * G], F32R, tag="w")
            nc.sync.dma_start(wtile[:], w[:])
            wttile = wpool.tile([C * (GT + 2), S, M * GT], F32R, tag="wt")
            nc.sync.dma_start(wttile[:], wt[:])

            for p in range(N_LOC // 2):
                n0 = 2 * p
                for b in range(NBLK + 1):
                    if b < NBLK:
                        i0, g, wsel = G * b, G, wtile
                    else:
                        i0, g, wsel = NBLK * G, GT, wttile
                    hh = g + 2
                    kk = C * hh
                    mm = M * g

                    xt = xpool.tile([kk, 2, W], F32R, tag="xt")
                    for img in range(2):
                        nc.sync.dma_start(
                            xt[:, img, :], x_v[:, i0 : i0 + hh, n0 + img, :]
                        )

                    ps = pspool.tile([mm, 2, WO], F32, tag="ps")
                    for s in range(S):
                        nc.tensor.matmul(
                            ps[:],
                            wsel[:, s, :],
                            xt[:, :, s : s + WO],
                            start=(s == 0),
                            stop=(s == S - 1),
                        )

                    ot = opool.tile([mm, 2, WO], F32, tag="ot")
                    nc.vector.tensor_copy(ot[:], ps[:])
                    for img in range(2):
                        nc.sync.dma_start(
                            y_v[:, i0 : i0 + g, n0 + img, :], ot[:, img, :]
                        )
    nc.compile()
    return nc


def _get_program():
    if "nc" not in _CACHE:
        _CACHE["nc"] = _build_program()
    return _CACHE["nc"]


def kernel(_input, _filter):
    x = _round_fp32r(np.asarray(_input, dtype=np.float32))
    f = np.asarray(_filter, dtype=np.float32)
    w_full = _round_fp32r(_toeplitz_weights(f, G))
    w_tail = _round_fp32r(_toeplitz_weights(f, GT))

    nc = _get_program()
    in_maps = [
        {"x": x[c * N_LOC : (c + 1) * N_LOC], "w": w_full, "wt": w_tail}
        for c in range(N_CORES)
    ]
    res = bass_utils.run_bass_kernel_spmd(nc, in_maps, core_ids=list(range(N_CORES)))
    return np.concatenate([r["y"] for r in res.results], axis=0)


# revision 11
# speedup vs baseline: 1.0632x; 1.0632x over previous
"""Trainium2 Bass kernel for 3x3 VALID conv (NCHW, stride 1) via block-Toeplitz GEMM.

Full input (64, 8, 256, 256) f32 + filter (8, 8, 3, 3) -> output (64, 8, 254, 254).
Data-parallel over batch: 8 images per NeuronCore, 8 cores.

Per-core formulation: for a block of G=14 output rows, load input rows
i0..i0+15 of all 8 channels into SBUF partitions (c*16+h) -> K=128.
For each kernel column offset s in {0,1,2} do one accumulating matmul with a
host-precomputed block-Toeplitz weight lhsT[c*16+h, m*14+i] = filter[m, c,
h-i, s], producing PSUM [112=(m,i), 2 images * 254 pixels = 508].  N=508 >=
256 so float32r (full fp32 precision) streams at 1 cycle/row on the PE.
"""

import numpy as np

import concourse.bacc as bacc
import concourse.bass as bass
import concourse.mybir as mybir
import concourse.tile as tile
from concourse import bass_utils

F32 = mybir.dt.float32
F32R = mybir.dt.float32r

N_CORES = 8
N_LOC = 8  # images per core
C, H, W = 8, 256, 256
M, R, S = 8, 3, 3
HO, WO = H - R + 1, W - S + 1  # 254, 254
G = 14  # output rows per full block
NBLK = 18  # full blocks (rows 0..251)
GT = 2  # tail block output rows (252, 253)

_CACHE = {}


def _round_fp32r(a):
    """Round fp32 array to float32r (12 explicit mantissa bits, RNE)."""
    a = np.ascontiguousarray(a, dtype=np.float32)
    try:
        from neuron_dtypes._impl import fp32r as _fp32r

        u = a.reshape(-1).view(np.uint32)
        r = np.asarray(_fp32r.cast_fp32_to_fp32r(len(u), u), np.uint32)
        return r.view(np.float32).reshape(a.shape)
    except Exception:
        u = a.reshape(-1).view(np.uint32).astype(np.uint64)
        lsb = (u >> 12) & 1
        r = ((u + 0x7FF + lsb) & 0xFFFFF000).astype(np.uint32)
        return r.view(np.float32).reshape(a.shape)


def _toeplitz_weights(f, g):
    """lhsT[c*(g+2)+h, s, m*g+i] = f[m, c, h-i, s] for 0 <= h-i < 3 else 0."""
    hh = g + 2
    out = np.zeros((C * hh, S, M * g), np.float32)
    for c in range(C):
        for h in range(hh):
            for m in range(M):
                for i in range(g):
                    r = h - i
                    if 0 <= r < R:
                        out[c * hh + h, :, m * g + i] = f[m, c, r, :]
    return out


def _build_program():
    nc = bacc.Bacc("TRN2", target_bir_lowering=False, debug=False)
    x = nc.dram_tensor("x", [N_LOC, C, H, W], F32R, kind="ExternalInput").ap()
    w = nc.dram_tensor("w", [C * (G + 2), S, M * G], F32R, kind="ExternalInput").ap()
    wt = nc.dram_tensor("wt", [C * (GT + 2), S, M * GT], F32R, kind="ExternalInput").ap()
    y = nc.dram_tensor("y", [N_LOC, M, HO, WO], F32, kind="ExternalOutput").ap()

    x_v = x.rearrange("n c h w -> c h n w")
    y_v = y.rearrange("n m i j -> m i n j")

    with tile.TileContext(nc) as tc:
        with (
            tc.tile_pool(name="wpool", bufs=1) as wpool,
            tc.tile_pool(name="xpool", bufs=8) as xpool,
            tc.tile_pool(name="opool", bufs=8) as opool,
            tc.tile_pool(name="psum", bufs=6, space=bass.MemorySpace.PSUM) as pspool,
        ):
            wtile = wpool.tile([C * (G + 2), S, M * G], F32R, tag="w")
            nc.sync.dma_start(wtile[:], w[:])
            wttile = wpool.tile([C * (GT + 2), S, M * GT], F32R, tag="wt")
            nc.sync.dma_start(wttile[:], wt[:])

            for p in range(N_LOC // 2):
                n0 = 2 * p
                for b in range(NBLK + 1):
                    t = p * (NBLK + 1) + b
                    # Alternate the two HWDGE rings (SP / ACT) between loads
                    # and stores so both descriptor-service paths run.
                    ld_eng = nc.sync if t % 2 == 0 else nc.scalar
                    st_eng = nc.scalar if t % 2 == 0 else nc.sync
                    if b < NBLK:
                        i0, g, wsel = G * b, G, wtile
                    else:
                        i0, g, wsel = NBLK * G, GT, wttile
                    hh = g + 2
                    kk = C * hh
                    mm = M * g

                    xt = xpool.tile([kk, 2, W], F32R, tag="xt")
                    for img in range(2):
                        ld_eng.dma_start(
                            xt[:, img, :], x_v[:, i0 : i0 + hh, n0 + img, :]
                        )

                    ps = pspool.tile([mm, 2, WO], F32, tag="ps")
                    for s in range(S):
                        nc.tensor.matmul(
                            ps[:],
                            wsel[:, s, :],
                            xt[:, :, s : s + WO],
                            start=(s == 0),
                            stop=(s == S - 1),
                        )

                    ot = opool.tile([mm, 2, WO], F32, tag="ot")
                    nc.vector.tensor_copy(ot[:], ps[:])
                    for img in range(2):
                        st_eng.dma_start(
                            y_v[:, i0 : i0 + g, n0 + img, :], ot[:, img, :]
                        )
    nc.compile()
    return nc


def _get_program():
    if "nc" not in _CACHE:
        _CACHE["nc"] = _build_program()
    return _CACHE["nc"]


def kernel(_input, _filter):
    x = _round_fp32r(np.asarray(_input, dtype=np.float32))
    f = np.asarray(_filter, dtype=np.float32)
    w_full = _round_fp32r(_toeplitz_weights(f, G))
    w_tail = _round_fp32r(_toeplitz_weights(f, GT))

    nc = _get_program()
    in_maps = [
        {"x": x[c * N_LOC : (c + 1) * N_LOC], "w": w_full, "wt": w_tail}
        for c in range(N_CORES)
    ]
    res = bass_utils.run_bass_kernel_spmd(nc, in_maps, core_ids=list(range(N_CORES)))
    return np.concatenate([r["y"] for r in res.results], axis=0)


# revision 14
# speedup vs baseline: 1.1512x; 1.0828x over previous
"""Trainium2 Bass kernel for 3x3 VALID conv (NCHW, stride 1) via block-Toeplitz GEMM.

Full input (64, 8, 256, 256) f32 + filter (8, 8, 3, 3) -> output (64, 8, 254, 254).
Data-parallel over batch: 8 images per NeuronCore, 8 cores.

Row-pair layout for DMA efficiency: each SBUF partition (c, hb) holds TWO
consecutive input rows (2 KB contiguous in DRAM -> 2 KB descriptors; a 256 KB
load instruction = 16 x 16KB packets engages all 16 SDMA engines).

Per block of 30 output rows: K = (8 ch) x (16 row-pairs) = 128 partitions.
Even output rows (i0+2q) accumulate in one PSUM tile, odd rows in another.
Taps split by row parity of the input row they touch:
  T2_s  = W[m,c,0,s]@[hb=q] + W[m,c,2,s]@[hb=q+1]   (same-parity taps)
  D1E_s = W[m,c,1,s]@[hb=q]                          (even-out, odd-row tap)
  D1O_s = W[m,c,1,s]@[hb=q+1]                        (odd-out, even-row tap)
  psE += sum_s T2_s @ X(par0, s) + D1E_s @ X(par1, s)
  psO += sum_s T2_s @ X(par1, s) + D1O_s @ X(par0, s)
N = 2 images x 254 pixels = 508 >= 256 so float32r (fp32 rounded to 12
mantissa bits) streams at 1 cycle/row on the PE.  Outputs are staged so each
SBUF partition holds an even+odd row pair -> 2 KB store descriptors.
"""

import numpy as np

import concourse.bacc as bacc
import concourse.bass as bass
import concourse.mybir as mybir
import concourse.tile as tile
from concourse import bass_utils

F32 = mybir.dt.float32
F32R = mybir.dt.float32r

N_CORES = 8
N_LOC = 8  # images per core
C, H, W = 8, 256, 256
MCH, R, S = 8, 3, 3
HO, WO = H - R + 1, W - S + 1  # 254, 254
Q = 15  # even/odd output row pairs per full block (30 rows)
NBLK = 8  # full blocks (rows 0..239)
QT = 7  # tail block row pairs (rows 240..253)

_CACHE = {}


def _round_fp32r(a):
    """Round fp32 array to float32r (12 explicit mantissa bits, RNE)."""
    a = np.ascontiguousarray(a, dtype=np.float32)
    try:
        from neuron_dtypes._impl import fp32r as _fp32r

        u = a.reshape(-1).view(np.uint32)
        r = np.asarray(_fp32r.cast_fp32_to_fp32r(len(u), u), np.uint32)
        return r.view(np.float32).reshape(a.shape)
    except Exception:
        u = a.reshape(-1).view(np.uint32).astype(np.uint64)
        lsb = (u >> 12) & 1
        r = ((u + 0x7FF + lsb) & 0xFFFFF000).astype(np.uint32)
        return r.view(np.float32).reshape(a.shape)


def _pair_weights(f, q_cnt):
    """w[kk, 9, mm]: 3 groups (T2, D1E, D1O) x 3 s, Toeplitz over row-pairs.

    w[c*(q_cnt+1)+hb, g*3+s, m*q_cnt+q]
    """
    hbn = q_cnt + 1
    kk = C * hbn
    mm = MCH * q_cnt
    out = np.zeros((kk, 9, mm), np.float32)
    for c in range(C):
        for m in range(MCH):
            for q in range(q_cnt):
                col = m * q_cnt + q
                for s in range(S):
                    # T2: r=0 at hb=q, r=2 at hb=q+1
                    out[c * hbn + q, 0 * 3 + s, col] += f[m, c, 0, s]
                    out[c * hbn + q + 1, 0 * 3 + s, col] += f[m, c, 2, s]
                    # D1E: r=1 at hb=q ; D1O: r=1 at hb=q+1
                    out[c * hbn + q, 1 * 3 + s, col] += f[m, c, 1, s]
                    out[c * hbn + q + 1, 2 * 3 + s, col] += f[m, c, 1, s]
    return out


def _build_program():
    nc = bacc.Bacc("TRN2", target_bir_lowering=False, debug=False)
    x = nc.dram_tensor("x", [N_LOC, C, H, W], F32R, kind="ExternalInput").ap()
    w = nc.dram_tensor("w", [C * (Q + 1), 9, MCH * Q], F32R, kind="ExternalInput").ap()
    wt = nc.dram_tensor(
        "wt", [C * (QT + 1), 9, MCH * QT], F32R, kind="ExternalInput"
    ).ap()
    y = nc.dram_tensor("y", [N_LOC, MCH, HO, WO], F32, kind="ExternalOutput").ap()

    # (c, hb) partitions; each holds rows 2hb, 2hb+1 of channel c (contiguous)
    x_v = x.rearrange("n c (hb pr) w -> c hb n (pr w)", pr=2)
    # output rows 2q,2q+1 of channel m stored from one partition (contiguous)
    y_v = y.rearrange("n m (q pr) j -> m q n (pr j)", pr=2)

    with tile.TileContext(nc) as tc:
        with (
            tc.tile_pool(name="wpool", bufs=1) as wpool,
            tc.tile_pool(name="xpool", bufs=6) as xpool,
            tc.tile_pool(name="opool", bufs=6) as opool,
            tc.tile_pool(name="psum", bufs=4, space=bass.MemorySpace.PSUM) as pspool,
        ):
            wtile = wpool.tile([C * (Q + 1), 9, MCH * Q], F32R, tag="w")
            nc.sync.dma_start(wtile[:], w[:])
            wttile = wpool.tile([C * (QT + 1), 9, MCH * QT], F32R, tag="wt")
            nc.sync.dma_start(wttile[:], wt[:])

            for p in range(N_LOC // 2):
                n0 = 2 * p
                for b in range(NBLK + 1):
                    t = p * (NBLK + 1) + b
                    ld_eng = nc.sync if t % 2 == 0 else nc.scalar
                    st_eng = nc.scalar if t % 2 == 0 else nc.sync
                    if b < NBLK:
                        i0, q_cnt, wsel = 2 * Q * b, Q, wtile
                    else:
                        i0, q_cnt, wsel = 2 * Q * NBLK, QT, wttile
                    hbn = q_cnt + 1
                    kk = C * hbn
                    mm = MCH * q_cnt
                    hb0 = i0 // 2

                    # [kk, img, 2 rows * W]; per img one 256KB DMA, 2KB descs
                    xt = xpool.tile([kk, 2, 2 * W], F32R, tag="xt")
                    for img in range(2):
                        ld_eng.dma_start(
                            xt[:, img, :], x_v[:, hb0 : hb0 + hbn, n0 + img, :]
                        )

                    psE = pspool.tile([mm, 2, WO], F32, tag="psE")
                    psO = pspool.tile([mm, 2, WO], F32, tag="psO")
                    # (group row in wtile, rhs row-parity offset, psum)
                    plan = [
                        (0, 0, psE, True),  # T2 @ par0 -> even
                        (1, 1, psE, False),  # D1E @ par1 -> even
                        (0, 1, psO, True),  # T2 @ par1 -> odd
                        (2, 0, psO, False),  # D1O @ par0 -> odd
                    ]
                    for gi, (g, par, ps, first) in enumerate(plan):
                        for s in range(S):
                            nc.tensor.matmul(
                                ps[:],
                                wsel[:kk, g * 3 + s, :],
                                xt[:, :, par * W + s : par * W + s + WO],
                                start=(first and s == 0),
                                stop=(not first and s == S - 1),
                            )

                    # stage even+odd rows adjacently: free = (img, par, j)
                    ot = opool.tile([mm, 2, 2, WO], F32, tag="ot")
                    nc.vector.tensor_copy(ot[:, :, 0, :], psE[:])
                    nc.vector.tensor_copy(ot[:, :, 1, :], psO[:])
                    q0 = i0 // 2
                    for img in range(2):
                        st_eng.dma_start(
                            y_v[:, q0 : q0 + q_cnt, n0 + img, :],
                            ot[:, img, :, :],
                        )
    nc.compile()
    return nc


def _get_program():
    if "nc" not in _CACHE:
        _CACHE["nc"] = _build_program()
    return _CACHE["nc"]


def kernel(_input, _filter):
    x = _round_fp32r(np.asarray(_input, dtype=np.float32))
    f = np.asarray(_filter, dtype=np.float32)
    w_full = _round_fp32r(_pair_weights(f, Q))
    w_tail = _round_fp32r(_pair_weights(f, QT))

    nc = _get_program()
    in_maps = [
        {"x": x[c * N_LOC : (c + 1) * N_LOC], "w": w_full, "wt": w_tail}
        for c in range(N_CORES)
    ]
    res = bass_utils.run_bass_kernel_spmd(nc, in_maps, core_ids=list(range(N_CORES)))
    return np.concatenate([r["y"] for r in res.results], axis=0)


# revision 15
# speedup vs baseline: 1.3468x; 1.1699x over previous
"""Trainium2 Bass kernel for 3x3 VALID conv (NCHW, stride 1) via block-Toeplitz GEMM.

Full input (64, 8, 256, 256) f32 + filter (8, 8, 3, 3) -> output (64, 8, 254, 254).
Data-parallel over batch: 8 images per NeuronCore, 8 cores.

Row-pair layout for DMA efficiency: each SBUF partition (c, hb) holds TWO
consecutive input rows (2 KB contiguous in DRAM -> 2 KB descriptors; a 256 KB
load instruction = 16 x 16KB packets engages all 16 SDMA engines).

Per block of 30 output rows: K = (8 ch) x (16 row-pairs) = 128 partitions.
Even output rows (i0+2q) accumulate in one PSUM tile, odd rows in another.
Taps split by row parity of the input row they touch:
  T2_s  = W[m,c,0,s]@[hb=q] + W[m,c,2,s]@[hb=q+1]   (same-parity taps)
  D1E_s = W[m,c,1,s]@[hb=q]                          (even-out, odd-row tap)
  D1O_s = W[m,c,1,s]@[hb=q+1]                        (odd-out, even-row tap)
  psE += sum_s T2_s @ X(par0, s) + D1E_s @ X(par1, s)
  psO += sum_s T2_s @ X(par1, s) + D1O_s @ X(par0, s)
N = 2 images x 254 pixels = 508 >= 256 so float32r (fp32 rounded to 12
mantissa bits) streams at 1 cycle/row on the PE.  Outputs are staged so each
SBUF partition holds an even+odd row pair -> 2 KB store descriptors.
"""

import numpy as np

import concourse.bacc as bacc
import concourse.bass as bass
import concourse.mybir as mybir
import concourse.tile as tile
from concourse import bass_utils

F32 = mybir.dt.float32
F32R = mybir.dt.float32r
BF16 = mybir.dt.bfloat16

N_CORES = 8
N_LOC = 8  # images per core
C, H, W = 8, 256, 256
MCH, R, S = 8, 3, 3
HO, WO = H - R + 1, W - S + 1  # 254, 254
Q = 15  # even/odd output row pairs per full block (30 rows)
NBLK = 8  # full blocks (rows 0..239)
QT = 7  # tail block row pairs (rows 240..253)

_CACHE = {}


def _round_fp32r(a):
    """Round fp32 array to float32r (12 explicit mantissa bits, RNE)."""
    a = np.ascontiguousarray(a, dtype=np.float32)
    try:
        from neuron_dtypes._impl import fp32r as _fp32r

        u = a.reshape(-1).view(np.uint32)
        r = np.asarray(_fp32r.cast_fp32_to_fp32r(len(u), u), np.uint32)
        return r.view(np.float32).reshape(a.shape)
    except Exception:
        u = a.reshape(-1).view(np.uint32).astype(np.uint64)
        lsb = (u >> 12) & 1
        r = ((u + 0x7FF + lsb) & 0xFFFFF000).astype(np.uint32)
        return r.view(np.float32).reshape(a.shape)


def _pair_weights(f, q_cnt):
    """w[kk, 9, mm]: 3 groups (T2, D1E, D1O) x 3 s, Toeplitz over row-pairs.

    w[c*(q_cnt+1)+hb, g*3+s, m*q_cnt+q]
    """
    hbn = q_cnt + 1
    kk = C * hbn
    mm = MCH * q_cnt
    out = np.zeros((kk, 9, mm), np.float32)
    for c in range(C):
        for m in range(MCH):
            for q in range(q_cnt):
                col = m * q_cnt + q
                for s in range(S):
                    # T2: r=0 at hb=q, r=2 at hb=q+1
                    out[c * hbn + q, 0 * 3 + s, col] += f[m, c, 0, s]
                    out[c * hbn + q + 1, 0 * 3 + s, col] += f[m, c, 2, s]
                    # D1E: r=1 at hb=q ; D1O: r=1 at hb=q+1
                    out[c * hbn + q, 1 * 3 + s, col] += f[m, c, 1, s]
                    out[c * hbn + q + 1, 2 * 3 + s, col] += f[m, c, 1, s]
    return out


def _build_program():
    nc = bacc.Bacc("TRN2", target_bir_lowering=False, debug=False)
    x = nc.dram_tensor("x", [N_LOC, C, H, W], BF16, kind="ExternalInput").ap()
    w = nc.dram_tensor("w", [C * (Q + 1), 9, MCH * Q], BF16, kind="ExternalInput").ap()
    wt = nc.dram_tensor(
        "wt", [C * (QT + 1), 9, MCH * QT], BF16, kind="ExternalInput"
    ).ap()
    y = nc.dram_tensor("y", [N_LOC, MCH, HO, WO], F32, kind="ExternalOutput").ap()

    # (c, hb) partitions; each holds rows 2hb, 2hb+1 of channel c (contiguous)
    x_v = x.rearrange("n c (hb pr) w -> c hb n (pr w)", pr=2)
    # output rows 2q,2q+1 of channel m stored from one partition (contiguous)
    y_v = y.rearrange("n m (q pr) j -> m q n (pr j)", pr=2)

    with tile.TileContext(nc) as tc:
        with (
            tc.tile_pool(name="wpool", bufs=1) as wpool,
            tc.tile_pool(name="xpool", bufs=6) as xpool,
            tc.tile_pool(name="opool", bufs=6) as opool,
            tc.tile_pool(name="psum", bufs=4, space=bass.MemorySpace.PSUM) as pspool,
        ):
            wtile = wpool.tile([C * (Q + 1), 9, MCH * Q], BF16, tag="w")
            nc.gpsimd.dma_start(wtile[:], w[:])
            wttile = wpool.tile([C * (QT + 1), 9, MCH * QT], BF16, tag="wt")
            nc.gpsimd.dma_start(wttile[:], wt[:])

            for p in range(N_LOC // 2):
                n0 = 2 * p
                for b in range(NBLK + 1):
                    t = p * (NBLK + 1) + b
                    ld_eng = nc.sync if t % 2 == 0 else nc.scalar
                    st_eng = nc.scalar if t % 2 == 0 else nc.sync
                    if b < NBLK:
                        i0, q_cnt, wsel = 2 * Q * b, Q, wtile
                    else:
                        i0, q_cnt, wsel = 2 * Q * NBLK, QT, wttile
                    hbn = q_cnt + 1
                    kk = C * hbn
                    mm = MCH * q_cnt
                    hb0 = i0 // 2

                    # [kk, img, 2 rows * W]; per img one 256KB DMA, 2KB descs
                    xt = xpool.tile([kk, 2, 2 * W], BF16, tag="xt")
                    for img in range(2):
                        ld_eng.dma_start(
                            xt[:, img, :], x_v[:, hb0 : hb0 + hbn, n0 + img, :]
                        )

                    psE = pspool.tile([mm, 2, WO], F32, tag="psE")
                    psO = pspool.tile([mm, 2, WO], F32, tag="psO")
                    # (group row in wtile, rhs row-parity offset, psum)
                    plan = [
                        (0, 0, psE, True),  # T2 @ par0 -> even
                        (1, 1, psE, False),  # D1E @ par1 -> even
                        (0, 1, psO, True),  # T2 @ par1 -> odd
                        (2, 0, psO, False),  # D1O @ par0 -> odd
                    ]
                    for gi, (g, par, ps, first) in enumerate(plan):
                        for s in range(S):
                            nc.tensor.matmul(
                                ps[:],
                                wsel[:kk, g * 3 + s, :],
                                xt[:, :, par * W + s : par * W + s + WO],
                                start=(first and s == 0),
                                stop=(not first and s == S - 1),
                            )

                    # stage even+odd rows adjacently: free = (img, par, j)
                    ot = opool.tile([mm, 2, 2, WO], F32, tag="ot")
                    nc.vector.tensor_copy(ot[:, :, 0, :], psE[:])
                    nc.vector.tensor_copy(ot[:, :, 1, :], psO[:])
                    q0 = i0 // 2
                    for img in range(2):
                        st_eng.dma_start(
                            y_v[:, q0 : q0 + q_cnt, n0 + img, :],
                            ot[:, img, :, :],
                        )
    nc.compile()
    return nc


def _get_program():
    if "nc" not in _CACHE:
        _CACHE["nc"] = _build_program()
    return _CACHE["nc"]


def _to_bf16(a):
    import ml_dtypes

    return np.ascontiguousarray(np.asarray(a, np.float32)).astype(ml_dtypes.bfloat16)


def kernel(_input, _filter):
    x = _to_bf16(_input)
    f = np.asarray(_filter, dtype=np.float32)
    w_full = _to_bf16(_pair_weights(f, Q))
    w_tail = _to_bf16(_pair_weights(f, QT))

    nc = _get_program()
    in_maps = [
        {"x": x[c * N_LOC : (c + 1) * N_LOC], "w": w_full, "wt": w_tail}
        for c in range(N_CORES)
    ]
    res = bass_utils.run_bass_kernel_spmd(nc, in_maps, core_ids=list(range(N_CORES)))
    return np.concatenate([r["y"] for r in res.results], axis=0)


# revision 16
# speedup vs baseline: 1.9297x; 1.4328x over previous
"""Trainium2 Bass kernel for 3x3 VALID conv (NCHW, stride 1) via block-Toeplitz GEMM.

Full input (64, 8, 256, 256) f32 + filter (8, 8, 3, 3) -> output (64, 8, 254, 254).
Data-parallel over batch: 8 images per NeuronCore, 8 cores.

Row-pair layout for DMA efficiency: each SBUF partition (c, hb) holds TWO
consecutive input rows (2 KB contiguous in DRAM -> 2 KB descriptors; a 256 KB
load instruction = 16 x 16KB packets engages all 16 SDMA engines).

Per block of 30 output rows: K = (8 ch) x (16 row-pairs) = 128 partitions.
Even output rows (i0+2q) accumulate in one PSUM tile, odd rows in another.
Taps split by row parity of the input row they touch:
  T2_s  = W[m,c,0,s]@[hb=q] + W[m,c,2,s]@[hb=q+1]   (same-parity taps)
  D1E_s = W[m,c,1,s]@[hb=q]                          (even-out, odd-row tap)
  D1O_s = W[m,c,1,s]@[hb=q+1]                        (odd-out, even-row tap)
  psE += sum_s T2_s @ X(par0, s) + D1E_s @ X(par1, s)
  psO += sum_s T2_s @ X(par1, s) + D1O_s @ X(par0, s)
N = 2 images x 254 pixels = 508 >= 256 so float32r (fp32 rounded to 12
mantissa bits) streams at 1 cycle/row on the PE.  Outputs are staged so each
SBUF partition holds an even+odd row pair -> 2 KB store descriptors.
"""

import numpy as np

import concourse.bacc as bacc
import concourse.bass as bass
import concourse.mybir as mybir
import concourse.tile as tile
from concourse import bass_utils

F32 = mybir.dt.float32
F32R = mybir.dt.float32r
BF16 = mybir.dt.bfloat16

N_CORES = 8
N_LOC = 8  # images per core
C, H, W = 8, 256, 256
MCH, R, S = 8, 3, 3
HO, WO = H - R + 1, W - S + 1  # 254, 254
Q = 15  # even/odd output row pairs per full block (30 rows)
NBLK = 8  # full blocks (rows 0..239)
QT = 7  # tail block row pairs (rows 240..253)

_CACHE = {}


def _round_fp32r(a):
    """Round fp32 array to float32r (12 explicit mantissa bits, RNE)."""
    a = np.ascontiguousarray(a, dtype=np.float32)
    try:
        from neuron_dtypes._impl import fp32r as _fp32r

        u = a.reshape(-1).view(np.uint32)
        r = np.asarray(_fp32r.cast_fp32_to_fp32r(len(u), u), np.uint32)
        return r.view(np.float32).reshape(a.shape)
    except Exception:
        u = a.reshape(-1).view(np.uint32).astype(np.uint64)
        lsb = (u >> 12) & 1
        r = ((u + 0x7FF + lsb) & 0xFFFFF000).astype(np.uint32)
        return r.view(np.float32).reshape(a.shape)


def _pair_weights(f, q_cnt):
    """w[kk, 9, mm]: 3 groups (T2, D1E, D1O) x 3 s, Toeplitz over row-pairs.

    w[c*(q_cnt+1)+hb, g*3+s, m*q_cnt+q]
    """
    hbn = q_cnt + 1
    kk = C * hbn
    mm = MCH * q_cnt
    out = np.zeros((kk, 9, mm), np.float32)
    for c in range(C):
        for m in range(MCH):
            for q in range(q_cnt):
                col = m * q_cnt + q
                for s in range(S):
                    # T2: r=0 at hb=q, r=2 at hb=q+1
                    out[c * hbn + q, 0 * 3 + s, col] += f[m, c, 0, s]
                    out[c * hbn + q + 1, 0 * 3 + s, col] += f[m, c, 2, s]
                    # D1E: r=1 at hb=q ; D1O: r=1 at hb=q+1
                    out[c * hbn + q, 1 * 3 + s, col] += f[m, c, 1, s]
                    out[c * hbn + q + 1, 2 * 3 + s, col] += f[m, c, 1, s]
    return out


def _build_program():
    nc = bacc.Bacc("TRN2", target_bir_lowering=False, debug=False)
    x = nc.dram_tensor("x", [N_LOC, C, H, W], BF16, kind="ExternalInput").ap()
    w = nc.dram_tensor("w", [C * (Q + 1), 9, MCH * Q], BF16, kind="ExternalInput").ap()
    wt = nc.dram_tensor(
        "wt", [C * (QT + 1), 9, MCH * QT], BF16, kind="ExternalInput"
    ).ap()
    y = nc.dram_tensor("y", [N_LOC, MCH, HO, WO], F32, kind="ExternalOutput").ap()

    # (c, hb) partitions; each holds rows 2hb, 2hb+1 of channel c (contiguous)
    x_v = x.rearrange("n c (hb pr) w -> c hb n (pr w)", pr=2)
    # output rows 2q,2q+1 of channel m stored from one partition (contiguous)
    y_v = y.rearrange("n m (q pr) j -> m q n (pr j)", pr=2)

    with tile.TileContext(nc) as tc:
        with (
            tc.tile_pool(name="wpool", bufs=1) as wpool,
            tc.tile_pool(name="xpool", bufs=6) as xpool,
            tc.tile_pool(name="opool", bufs=6) as opool,
            tc.tile_pool(name="psum", bufs=4, space=bass.MemorySpace.PSUM) as pspool,
        ):
            wtile = wpool.tile([C * (Q + 1), 9, MCH * Q], BF16, tag="w")
            nc.gpsimd.dma_start(wtile[:], w[:])
            wttile = wpool.tile([C * (QT + 1), 9, MCH * QT], BF16, tag="wt")
            nc.gpsimd.dma_start(wttile[:], wt[:])

            for p in range(N_LOC // 2):
                n0 = 2 * p
                for b in range(NBLK + 1):
                    t = p * (NBLK + 1) + b
                    ld_eng = nc.sync if t % 2 == 0 else nc.scalar
                    st_eng = nc.scalar if t % 2 == 0 else nc.sync
                    if b < NBLK:
                        i0, q_cnt, wsel = 2 * Q * b, Q, wtile
                    else:
                        i0, q_cnt, wsel = 2 * Q * NBLK, QT, wttile
                    hbn = q_cnt + 1
                    kk = C * hbn
                    mm = MCH * q_cnt
                    hb0 = i0 // 2

                    # [kk, img, 2 rows * W]; per img one 256KB DMA, 2KB descs
                    xt = xpool.tile([kk, 2, 2 * W], BF16, tag="xt")
                    for img in range(2):
                        ld_eng.dma_start(
                            xt[:, img, :], x_v[:, hb0 : hb0 + hbn, n0 + img, :]
                        )

                    psE = pspool.tile([mm, 2, WO], F32, tag="psE")
                    psO = pspool.tile([mm, 2, WO], F32, tag="psO")
                    # (group row in wtile, rhs row-parity offset, psum)
                    plan = [
                        (0, 0, psE, True),  # T2 @ par0 -> even
                        (1, 1, psE, False),  # D1E @ par1 -> even
                        (0, 1, psO, True),  # T2 @ par1 -> odd
                        (2, 0, psO, False),  # D1O @ par0 -> odd
                    ]
                    for gi, (g, par, ps, first) in enumerate(plan):
                        for s in range(S):
                            nc.tensor.matmul(
                                ps[:],
                                wsel[:kk, g * 3 + s, :],
                                xt[:, :, par * W + s : par * W + s + WO],
                                start=(first and s == 0),
                                stop=(not first and s == S - 1),
                            )

                    # stage even+odd rows adjacently: free = (img, par, j)
                    ot = opool.tile([mm, 2, 2, WO], F32, tag="ot")
                    nc.vector.tensor_copy(ot[:, :, 0, :], psE[:])
                    nc.vector.tensor_copy(ot[:, :, 1, :], psO[:])
                    q0 = i0 // 2
                    for img in range(2):
                        # SWDGE (Q0) spreads across all 16 SDMA engines,
                        # including E72-79 that the HWDGE rings can't reach.
                        nc.gpsimd.dma_start(
                            y_v[:, q0 : q0 + q_cnt, n0 + img, :],
                            ot[:, img, :, :],
                        )
    nc.compile()
    return nc


def _get_program():
    if "nc" not in _CACHE:
        _CACHE["nc"] = _build_program()
    return _CACHE["nc"]


def _to_bf16(a):
    import ml_dtypes

    return np.ascontiguousarray(np.asarray(a, np.float32)).astype(ml_dtypes.bfloat16)


def kernel(_input, _filter):
    x = _to_bf16(_input)
    f = np.asarray(_filter, dtype=np.float32)
    w_full = _to_bf16(_pair_weights(f, Q))
    w_tail = _to_bf16(_pair_weights(f, QT))

    nc = _get_program()
    in_maps = [
        {"x": x[c * N_LOC : (c + 1) * N_LOC], "w": w_full, "wt": w_tail}
        for c in range(N_CORES)
    ]
    res = bass_utils.run_bass_kernel_spmd(nc, in_maps, core_ids=list(range(N_CORES)))
    return np.concatenate([r["y"] for r in res.results], axis=0)
